# revision 11
# baseline (speedup 1.0000x reference)
"""Trainium2 Bass kernel for nn_AbsSeq2SeqLSTM (bi-LSTM encoder + greedy-argmax LSTM decoder).

Sharding: pure data parallelism - batch 1024 split as 128 per NeuronCore across 8 cores;
all weights replicated.

Key structure (per core, B_local=128, S=128, H=512, T=48):
  - All four gates through ONE sigmoid lookup: tanh(x) = 2*sigmoid(2x)-1, with the
    g-gate's weight columns pre-scaled by 2 on host. Gate blocks laid out (i,g | f,o)
    in two [128,1024] PSUM chunks per direction so ScalarE runs 2 big ACTs per step.
  - Input projections folded on host into per-class tables; one-hot tables built on
    host with 4-band replication so the 4 per-gate one-hot matmuls run concurrently
    in disjoint PE row groups (tile_position row tiling).
  - Encoder recurrent matmuls in fp8e4 DoubleRow (K=256 per MM, 8 MMs/step/dir);
    decoder in bf16 (latency-bound, keeps logits/argmax accurate).
  - Gates/cell state in bf16 (DVE 2x mode); per-step h transpose on TensorE into a
    PSUM slot recycled from the gate chunk, then cast to fp8 (enc) / bf16 (dec).
"""

import os
import sys

for _p in ("/opt/trn_rl_repo", "/root/.axon_site/_ro/trn_rl_repo"):
    if os.path.isdir(_p) and _p not in sys.path:
        sys.path.append(_p)

import numpy as np
import ml_dtypes
import concourse.bass as bass
import concourse.tile as tile
from concourse import bacc, mybir
from concourse.bass_utils import run_bass_kernel_spmd

N_CORES = 8
B_LOC = 128
H = 512
G = 2048
VIN = 16
VOUT = 9

f32 = mybir.dt.float32
bf16 = mybir.dt.bfloat16
f8 = mybir.dt.float8e4
i32 = mybir.dt.int32
u32 = mybir.dt.uint32
AF = mybir.ActivationFunctionType
ALU = mybir.AluOpType
AX = mybir.AxisListType
DR = mybir.MatmulPerfMode.DoubleRow

ENC_FP8 = os.environ.get("K_ENC_FP8", "1") == "1"
DEC_FP8 = os.environ.get("K_DEC_FP8", "1") == "1"
ehdt = f8 if ENC_FP8 else bf16

np_bf16 = ml_dtypes.bfloat16
np_f8 = ml_dtypes.float8_e4m3

# device gate order: chunk A = (i, g), chunk B = (f, o); reference order i,f,g,o
GPERM = [0, 2, 1, 3]  # device block k holds reference gate GPERM[k]


def build_program(S=128, T=48):
    nc = bacc.Bacc("TRN2", target_bir_lowering=False, debug=False)

    OHTS = nc.dram_tensor("ohts", [B_LOC, S * 128], bf16, kind="ExternalInput").ap()
    WH8 = {
        "f": nc.dram_tensor("wh8f", [128, 2 * 2 * G], f8, kind="ExternalInput").ap(),
        "b": nc.dram_tensor("wh8b", [128, 2 * 2 * G], f8, kind="ExternalInput").ap(),
        "d": nc.dram_tensor("wh8d", [128, 2 * 2 * G], f8, kind="ExternalInput").ap(),
    }
    WH16 = {
        "f": nc.dram_tensor("wh16f", [128, 4 * G], bf16, kind="ExternalInput").ap(),
        "b": nc.dram_tensor("wh16b", [128, 4 * G], bf16, kind="ExternalInput").ap(),
        "d": nc.dram_tensor("wh16d", [128, 4 * G], bf16, kind="ExternalInput").ap(),
    }
    ZX = {
        "f": nc.dram_tensor("zxf", [128, G], bf16, kind="ExternalInput").ap(),
        "b": nc.dram_tensor("zxb", [128, G], bf16, kind="ExternalInput").ap(),
    }
    ZED = nc.dram_tensor("zed", [128, G], bf16, kind="ExternalInput").ap()
    WOUT = nc.dram_tensor("wout", [128, 64], bf16, kind="ExternalInput").ap()
    BOUT = nc.dram_tensor("bout", [1, 16], bf16, kind="ExternalInput").ap()
    IDENT = nc.dram_tensor("ident", [128, 128], bf16, kind="ExternalInput").ap()
    IOTA9 = nc.dram_tensor("iota9", [128, 128], f32, kind="ExternalInput").ap()
    ONES = nc.dram_tensor("ones", [1, 128], bf16, kind="ExternalInput").ap()
    OH0 = nc.dram_tensor("oh0", [128, 128], bf16, kind="ExternalInput").ap()
    PREDS = nc.dram_tensor("preds", [B_LOC, T * VOUT], f32, kind="ExternalOutput").ap()

    with tile.TileContext(nc) as tc:
        _emit(nc, tc, S, T, OHTS, WH8, WH16, ZX, ZED, WOUT, BOUT, IDENT, IOTA9, ONES, OH0, PREDS)
    nc.compile()
    return nc


def _emit(nc, tc, S, T, OHTS, WH8, WH16, ZX, ZED, WOUT, BOUT, IDENT, IOTA9, ONES, OH0, PREDS):
    from contextlib import ExitStack

    ctx = ExitStack()
    const = ctx.enter_context(tc.tile_pool(name="const", bufs=1))

    # ---------------- phase 0: constants into SBUF (all pre-converted on host) --------
    ohts = const.tile([128, S * 128], bf16, name="ohts_sb")
    CH = 8
    chw = S * 128 // CH
    order = [0, CH - 1, 1, CH - 2, 2, CH - 3, 3, CH - 4][:CH]
    for k in order:
        nc.sync.dma_start(ohts[:, k * chw : (k + 1) * chw], OHTS[:, k * chw : (k + 1) * chw])

    zxr = {}
    for d in ("f", "b"):
        zt = const.tile([128, G], bf16, name=f"zxr{d}")
        nc.sync.dma_start(zt[:], ZX[d][:])
        zxr[d] = zt
    wh8 = {}
    f8dirs = (("f", "b", "d") if DEC_FP8 else ("f", "b")) if ENC_FP8 else ()
    for d in f8dirs:
        wt = const.tile([128, 2 * 2 * G], f8, name=f"wh8{d}")
        nc.sync.dma_start(wt[:], WH8[d][:])
        wh8[d] = wt
    wh16 = {}
    for d in ("f", "b", "d"):
        if d in f8dirs:
            continue
        wt = const.tile([128, 4 * G], bf16, name=f"wh16{d}")
        nc.sync.dma_start(wt[:], WH16[d][:])
        wh16[d] = wt
    zed_sb = const.tile([128, G], bf16, name="zed_sb")
    nc.sync.dma_start(zed_sb[:], ZED[:])
    wout_sb = const.tile([128, 64], bf16, name="wout_sb")
    nc.sync.dma_start(wout_sb[:], WOUT[:])
    bout_sb = const.tile([1, 16], bf16, name="bout_sb")
    nc.sync.dma_start(bout_sb[:], BOUT[:])
    ident_sb = const.tile([128, 128], bf16, name="ident_sb")
    nc.sync.dma_start(ident_sb[:], IDENT[:])
    iota9_sb = const.tile([128, 128], f32, name="iota9_sb")
    nc.sync.dma_start(iota9_sb[:], IOTA9[:])
    ones_sb = const.tile([1, 128], bf16, name="ones_sb")
    nc.sync.dma_start(ones_sb[:], ONES[:])
    oh0_sb = const.tile([128, 128], bf16, name="oh0_sb")
    nc.sync.dma_start(oh0_sb[:], OH0[:])

    # ---------------- PSUM pools: 2 chunks x [128,1024] f32 per direction = 8 banks ---
    pz = ctx.enter_context(tc.tile_pool(name="pz", bufs=1, space="PSUM"))
    # SBUF pools
    pg = ctx.enter_context(tc.tile_pool(name="pg", bufs=2))
    pc = ctx.enter_context(tc.tile_pool(name="pc", bufs=2))
    ph = ctx.enter_context(tc.tile_pool(name="ph", bufs=2))

    def alloc_z(d, t):
        dtag = "f" if d == "d" else d
        zA = pz.tile([128, 1024], f32, tag=f"zA{dtag}", name=f"zA{d}{t}")
        zB = pz.tile([128, 1024], f32, tag=f"zB{dtag}", name=f"zB{d}{t}")
        return zA, zB

    def zslice(zA, zB, k):
        # device gate block k: 0=i (A lo), 1=g (A hi), 2=f (B lo), 3=o (B hi)
        zc = zA if k < 2 else zB
        return zc[:, (k % 2) * 512 : (k % 2) * 512 + 512]

    def enc_onehots(d, t, tok_step, zA, zB, only):
        # 4 concurrent one-hot matmuls, one per gate block, in disjoint row bands
        for k in range(4):
            nc.tensor.matmul(
                zslice(zA, zB, k),
                ohts[32 * k : 32 * k + VIN, tok_step * 128 : (tok_step + 1) * 128],
                zxr[d][32 * k : 32 * k + VIN, k * 512 : (k + 1) * 512],
                start=True,
                stop=only,
                tile_position=(32 * k, 0),
            )

    def enc_mains_fp8(d, t, zA, zB, hT, opener=False):
        for k in range(4):
            dst = zslice(zA, zB, k)
            for i in range(2):
                lhsT = hT[:, 256 * i : 256 * i + 256].rearrange("p (j m) -> p j m", j=2)
                rhs = wh8[d][:, 4096 * i + 2048 * 0 : 4096 * i + 4096].rearrange(
                    "p (j n) -> p j n", j=2
                )[:, :, k * 512 : (k + 1) * 512]
                nc.tensor.matmul(
                    dst, lhsT, rhs, start=(opener and i == 0), stop=(i == 1), perf_mode=DR
                )

    def mains_bf16(d, t, zA, zB, hT, wtile, opener=False):
        for k in range(4):
            dst = zslice(zA, zB, k)
            for kc in range(4):
                nc.tensor.matmul(
                    dst,
                    hT[:, kc * 128 : (kc + 1) * 128],
                    wtile[:, kc * G + k * 512 : kc * G + (k + 1) * 512],
                    start=(opener and kc == 0),
                    stop=(kc == 3),
                )

    def sig(d, t, zA, zB):
        """Two sigmoid chunks: gA = sigma(i,g pre-acts), gB = sigma(f,o)."""
        dtag = "f" if d == "d" else d
        gA = pg.tile([128, 1024], bf16, tag=f"gA{dtag}", name=f"gA{d}{t}")
        gB = pg.tile([128, 1024], bf16, tag=f"gB{dtag}", name=f"gB{d}{t}")
        nc.scalar.activation(gA[:], zA[:], AF.Sigmoid)
        nc.scalar.activation(gB[:], zB[:], AF.Sigmoid)
        return gA, gB

    def cell(d, t, gA, gB, c_prev, first):
        """DVE cell update -> c2 (bf16)."""
        dtag = "f" if d == "d" else d
        g2 = pg.tile([128, 512], bf16, tag=f"g2{dtag}", name=f"g2{d}{t}")
        nc.vector.tensor_scalar(g2[:], gA[:, 512:1024], 2.0, 1.0, ALU.mult, ALU.subtract)
        c_new = pc.tile([128, 512], bf16, tag=f"c{dtag}", name=f"c{d}{t}")
        if first:
            nc.vector.tensor_tensor(c_new[:], gA[:, 0:512], g2[:], ALU.mult)
        else:
            t1 = pg.tile([128, 512], bf16, tag=f"t1{dtag}", name=f"t1{d}{t}")
            nc.vector.tensor_tensor(t1[:], gA[:, 0:512], g2[:], ALU.mult)
            nc.vector.tensor_tensor(c_new[:], gB[:, 0:512], c_prev[:], ALU.mult)
            nc.vector.tensor_tensor(c_new[:], c_new[:], t1[:], ALU.add)
        return c_new

    def tr_o(d, t, gB):
        """Transpose the o gate into the zB PSUM slot (free after sigma read it)."""
        dtag = "f" if d == "d" else d
        trPo = pz.tile([128, 512], bf16, tag=f"zB{dtag}", name=f"tro{d}{t}")
        for kc in range(4):
            nc.tensor.transpose(
                trPo[:, kc * 128 : (kc + 1) * 128],
                gB[:, 512 + kc * 128 : 512 + (kc + 1) * 128],
                ident_sb[:],
            )
        return trPo

    def tr_c(d, t, c_new):
        dtag = "f" if d == "d" else d
        trPc = pz.tile([128, 512], bf16, tag=f"zA{dtag}", name=f"trc{d}{t}")
        for kc in range(4):
            nc.tensor.transpose(
                trPc[:, kc * 128 : (kc + 1) * 128], c_new[:, kc * 128 : (kc + 1) * 128], ident_sb[:]
            )
        return trPc

    def fin(d, t, trPo, trPc, out_dt):
        """tanh on transposed cell, then hT = oT * tanh(cT) directly in hT layout."""
        dtag = "f" if d == "d" else d
        tcT = pg.tile([128, 512], bf16, tag=f"tc{dtag}", name=f"tc{d}{t}")
        nc.scalar.activation(tcT[:], trPc[:], AF.Tanh)
        hT_new = ph.tile([128, 512], out_dt, tag=f"hT{dtag}{out_dt}", name=f"hT{d}{t}")
        nc.vector.tensor_tensor(hT_new[:], trPo[:], tcT[:], ALU.mult)
        return hT_new

    # ---------------- encoder ----------------------------------------------------------
    cs = {"f": None, "b": None}
    hTs = {"f": None, "b": None}
    for t in range(S):
        zf = alloc_z("f", t)
        enc_onehots("f", t, t, *zf, only=(t == 0))
        if t > 0:
            if ENC_FP8:
                enc_mains_fp8("f", t, *zf, hTs["f"])
            else:
                mains_bf16("f", t, *zf, hTs["f"], wh16["f"])
        zb = alloc_z("b", t)
        enc_onehots("b", t, S - 1 - t, *zb, only=(t == 0))
        if t > 0:
            if ENC_FP8:
                enc_mains_fp8("b", t, *zb, hTs["b"])
            else:
                mains_bf16("b", t, *zb, hTs["b"], wh16["b"])
        # breadth-first tails so f/b alternate in each engine's FIFO
        gAf, gBf = sig("f", t, *zf)
        gAb, gBb = sig("b", t, *zb)
        cs["f"] = cell("f", t, gAf, gBf, cs["f"], t == 0)
        cs["b"] = cell("b", t, gAb, gBb, cs["b"], t == 0)
        trPo_f = tr_o("f", t, gBf)
        trPc_f = tr_c("f", t, cs["f"])
        trPo_b = tr_o("b", t, gBb)
        trPc_b = tr_c("b", t, cs["b"])
        hTs["f"] = fin("f", t, trPo_f, trPc_f, ehdt if t < S - 1 else bf16)
        hTs["b"] = fin("b", t, trPo_b, trPc_b, ehdt if t < S - 1 else bf16)

    # decoder init: sum of final fwd/bwd states (both bf16 casts on the last step)
    dhdt = f8 if (ENC_FP8 and DEC_FP8) else bf16
    c_d = pc.tile([128, 512], bf16, tag="cf", name="cd_init")
    nc.vector.tensor_tensor(c_d[:], cs["f"][:], cs["b"][:], ALU.add)
    hT_d = ph.tile([128, 512], dhdt, tag=f"hTf{dhdt}", name="hTd_init")
    nc.vector.tensor_tensor(hT_d[:], hTs["f"][:], hTs["b"][:], ALU.add)

    # ---------------- decoder ----------------------------------------------------------
    logits_all = const.tile([B_LOC, T * VOUT], f32, name="logits_all")
    psm = ctx.enter_context(tc.tile_pool(name="psm", bufs=2))

    ohT4 = oh0_sb
    prev_ohp = None
    for t in range(T):
        zA, zB = alloc_z("d", t)
        # mains first so the PE isn't head-of-line blocked on the argmax chain
        if ENC_FP8 and DEC_FP8:
            enc_mains_fp8("d", t, zA, zB, hT_d, opener=True)
        else:
            mains_bf16("d", t, zA, zB, hT_d, wh16["d"], opener=True)
        junk = pz.tile([128, 512], bf16, tag="zAb", name=f"junk{t}")
        if prev_ohp is not None:
            nc.tensor.transpose(junk[:, 0:128], prev_ohp[:], ident_sb[:])
        junk2 = pz.tile([128, 512], f32, tag="zAb", name=f"junk2{t}")

        def jmm(stat):
            nc.tensor.matmul(junk2[:], stat, zxr["f"][:, 0:512], start=True, stop=True)

        for _ in range(4):  # no-dep fillers: run right after the mains
            jmm(ident_sb[:])
        for k in range(4):
            nc.tensor.matmul(
                zslice(zA, zB, k),
                ohT4[32 * k : 32 * k + VOUT, :],
                zed_sb[32 * k : 32 * k + VOUT, k * 512 : (k + 1) * 512],
                start=False,
                stop=True,
                tile_position=(32 * k, 0),
            )
        gA, gB = sig("d", t, zA, zB)
        # HAM-warming work with staggered real deps (outputs unused): keeps the
        # PE busy through the serial argmax/cell chain so it stays at 2.4 GHz
        nc.tensor.transpose(junk[:, 128:256], gA[:, 0:128], ident_sb[:])
        jmm(gA[:, 0:128])
        jmm(gA[:, 128:256])
        c_d = cell("d", t, gA, gB, c_d, False)
        nc.tensor.transpose(junk[:, 256:384], gB[:, 0:128], ident_sb[:])
        jmm(gB[:, 0:128])
        jmm(gB[:, 128:256])
        trPo = tr_o("d", t, gB)
        trPc = tr_c("d", t, c_d)
        nc.tensor.transpose(junk[:, 384:512], c_d[:, 0:128], ident_sb[:])
        jmm(c_d[:, 0:128])
        jmm(c_d[:, 128:256])
        hT_d = fin("d", t, trPo, trPc, dhdt)
        jmm(hT_d[:, 0:128] if dhdt == bf16 else c_d[:, 256:384])
        jmm(c_d[:, 384:512])

        lgP = pz.tile([128, 16], f32, tag="zAb", name=f"lg{t}")
        nc.tensor.matmul(lgP[:], ones_sb[0:1, :], bout_sb[0:1, :], start=True, stop=False)
        for kc in range(4):
            nc.tensor.matmul(
                lgP[:],
                hT_d[:, kc * 128 : (kc + 1) * 128],
                wout_sb[:, kc * 16 : (kc + 1) * 16],
                start=False,
                stop=(kc == 3),
            )
        lg_sb = logits_all[:, t * VOUT : (t + 1) * VOUT]
        nc.vector.tensor_copy(lg_sb, lgP[:, 0:VOUT])

        if t < T - 1:
            lmax = psm.tile([128, 8], f32, tag="lmax", name=f"lmax{t}")
            nc.vector.max(lmax[:], lg_sb)
            yidx = psm.tile([128, 8], u32, tag="yidx", name=f"yidx{t}")
            nc.vector.max_index(yidx[:], lmax[:], lg_sb)
            yf = psm.tile([128, 1], f32, tag="yf", name=f"yf{t}")
            nc.vector.tensor_copy(yf[:], yidx[:, 0:1])
            # 4-band one-hot in [v, b] layout via one PE transpose
            ohp4 = psm.tile([128, 128], bf16, tag="ohp", name=f"ohp{t}")
            nc.vector.tensor_scalar(ohp4[:], iota9_sb[:], yf[:, 0:1], None, ALU.is_equal)
            prev_ohp = ohp4
            trOH = pz.tile([128, 128], bf16, tag="zBb", name=f"troh{t}")
            nc.tensor.transpose(trOH[:], ohp4[:], ident_sb[:])
            ohT4 = psm.tile([128, 128], bf16, tag="ohT", name=f"ohT{t}")
            nc.vector.tensor_copy(ohT4[:], trOH[:])

    # ---------------- batched softmax over all logits -----------------------------------
    preds_sb = const.tile([B_LOC, T * VOUT], f32, name="preds_sb")
    exps = const.tile([B_LOC, T * VOUT], f32, name="exps")
    lmax48 = const.tile([128, T], f32, name="lmax48")
    lview = logits_all[:].rearrange("p (t v) -> p t v", v=VOUT)
    nc.vector.tensor_reduce(lmax48[:], lview, AX.X, ALU.max)
    for t in range(T):
        nc.vector.tensor_scalar(
            exps[:, t * VOUT : (t + 1) * VOUT],
            logits_all[:, t * VOUT : (t + 1) * VOUT],
            lmax48[:, t : t + 1],
            None,
            ALU.subtract,
        )
    nc.scalar.activation(exps[:], exps[:], AF.Exp)
    sums48 = const.tile([128, T], f32, name="sums48")
    eview = exps[:].rearrange("p (t v) -> p t v", v=VOUT)
    nc.vector.tensor_reduce(sums48[:], eview, AX.X, ALU.add)
    rec48 = const.tile([128, T], f32, name="rec48")
    nc.vector.reciprocal(rec48[:], sums48[:])
    for t in range(T):
        nc.vector.tensor_scalar(
            preds_sb[:, t * VOUT : (t + 1) * VOUT],
            exps[:, t * VOUT : (t + 1) * VOUT],
            rec48[:, t : t + 1],
            None,
            ALU.mult,
        )
    nc.sync.dma_start(PREDS[:], preds_sb[:])
    ctx.close()


_PROGRAM_CACHE = {}


def _get_program(S=128, T=48):
    key = (S, T)
    if key not in _PROGRAM_CACHE:
        _PROGRAM_CACHE[key] = build_program(S, T)
    return _PROGRAM_CACHE[key]


def _perm_scale(W):
    """Reorder gate blocks (i,f,g,o)->(i,g,f,o) and scale the g block by 2."""
    Hd = W.shape[0]
    W4 = np.asarray(W, np.float64).reshape(Hd, 4, 512)
    out = np.empty_like(W4)
    out[:, 0] = W4[:, 0]
    out[:, 1] = 2.0 * W4[:, 2]
    out[:, 2] = W4[:, 1]
    out[:, 3] = W4[:, 3]
    return out.reshape(Hd, 2048)


def _wh8_arrange(W):
    # W: [512, 2048] f64 -> [128, 2*2*2048] fp8 with value at col (i*4096+j*2048+n)
    # = W[256i+128j+ki, n]
    W4 = W.reshape(2, 2, 128, 2048)  # [i, j, ki, n]
    arr = np.ascontiguousarray(np.transpose(W4, (2, 0, 1, 3)).reshape(128, 8192))
    return arr.astype(np_f8)


def _wh16_arrange(W):
    # W: [512, 2048] -> [128, 4*2048] bf16 with value at col (kc*2048+n) = W[kc*128+p, n]
    return np.ascontiguousarray(W.reshape(4, 128, 2048).transpose(1, 0, 2).reshape(128, 8192)).astype(np_bf16)


def _zx_rep(zx):
    # [16, 2048] -> [128, 2048] bf16 with the table replicated at rows 32j+v
    out = np.zeros((128, 2048), np.float64)
    for j in range(4):
        out[32 * j : 32 * j + VIN] = zx
    return out.astype(np_bf16)


def _onehot_table(tokens, S):
    # [128, S*128] bf16: row 32j+v, col t*128+b = (tokens[b, t] == v)
    oh = (np.arange(VIN)[:, None, None] == tokens.T[None, :, :])  # [16, S, 128b]
    arr = np.zeros((32, S, 128), np.float32)
    arr[:VIN] = oh
    flat = arr.reshape(32, S * 128)
    return np.tile(flat, (4, 1)).astype(np_bf16)


def make_in_maps(tokens, Wh_f, Wh_b, Wh_d, zx_f, zx_b, ze_d, W_out, b_out):
    B = tokens.shape[0]
    assert B % N_CORES == 0
    bl = B // N_CORES
    ident = np.eye(128, dtype=np.float32).astype(np_bf16)
    iota9 = np.tile(np.tile(np.arange(32, dtype=np.float32), 4), (128, 1))
    ones = np.ones((1, 128), np.float32).astype(np_bf16)
    oh0 = np.zeros((128, 128), np.float32)
    oh0[::32, :] = 1.0
    oh0 = oh0.astype(np_bf16)

    whf = _perm_scale(Wh_f)
    whb = _perm_scale(Wh_b)
    whd = _perm_scale(Wh_d)
    zedp = np.zeros((128, 2048), np.float64)
    zep = _perm_scale(ze_d)
    for j in range(4):
        zedp[32 * j : 32 * j + VOUT] = zep

    wout = np.zeros((128, 64), np.float32)
    Wov = np.asarray(W_out, np.float32).reshape(4, 128, VOUT)
    for kc in range(4):
        wout[:, kc * 16 : kc * 16 + VOUT] = Wov[kc]

    common = dict(
        wh16d=_wh16_arrange(whd),
        zxf=_zx_rep(_perm_scale(zx_f)[:VIN]),
        zxb=_zx_rep(_perm_scale(zx_b)[:VIN]),
        zed=zedp.astype(np_bf16),
        wout=wout.astype(np_bf16),
        bout=np.pad(np.asarray(b_out, np.float32).reshape(1, VOUT), ((0, 0), (0, 16 - VOUT))).astype(np_bf16),
        ident=ident, iota9=iota9, ones=ones, oh0=oh0,
    )
    # the program declares all weight tensors; feed them all (unused ones are
    # never DMA'd by the device program)
    common["wh8f"] = _wh8_arrange(whf)
    common["wh8b"] = _wh8_arrange(whb)
    common["wh8d"] = _wh8_arrange(whd)
    common["wh16f"] = _wh16_arrange(whf)
    common["wh16b"] = _wh16_arrange(whb)
    S = tokens.shape[1]
    return [
        {**common, "ohts": _onehot_table(tokens[c * bl : (c + 1) * bl], S)}
        for c in range(N_CORES)
    ]


def fold_tables(emb_in, Wi_f, b_f, Wi_b, b_b, emb_out, Wi_d, b_d):
    f8_ = lambda x: np.asarray(x, np.float64)
    zx_f = f8_(emb_in) @ f8_(Wi_f) + f8_(b_f)
    zx_b = f8_(emb_in) @ f8_(Wi_b) + f8_(b_b)
    ze_d = f8_(emb_out) @ f8_(Wi_d) + f8_(b_d)
    return zx_f, zx_b, ze_d


def kernel(tokens, emb_in, Wi_f, Wh_f, b_f, Wi_b, Wh_b, b_b,
           emb_out, Wi_d, Wh_d, b_d, W_out, b_out, max_length):
    T = int(max_length)
    tokens = np.asarray(tokens, np.int32)
    B, S = tokens.shape
    zx_f, zx_b, ze_d = fold_tables(emb_in, Wi_f, b_f, Wi_b, b_b, emb_out, Wi_d, b_d)
    nc = _get_program(S, T)
    in_maps = make_in_maps(tokens, Wh_f, Wh_b, Wh_d, zx_f, zx_b, ze_d, W_out, b_out)
    res = run_bass_kernel_spmd(nc, in_maps, list(range(N_CORES)))
    bl = B // N_CORES
    preds = np.concatenate(
        [res.results[c]["preds"].reshape(bl, T, VOUT) for c in range(N_CORES)], axis=0
    )
    return np.ascontiguousarray(preds, np.float32)


# revision 12
# speedup vs baseline: 1.2207x; 1.2207x over previous
"""Trainium2 Bass kernel for nn_AbsSeq2SeqLSTM (bi-LSTM encoder + greedy-argmax LSTM decoder).

Sharding: pure data parallelism - batch 1024 split as 128 per NeuronCore across 8 cores;
all weights replicated.

Key structure (per core, B_local=128, S=128, H=512, T=48):
  - All four gates through ONE sigmoid lookup: tanh(x) = 2*sigmoid(2x)-1, with the
    g-gate's weight columns pre-scaled by 2 on host. Gate blocks laid out (i,g | f,o)
    in two [128,1024] PSUM chunks per direction so ScalarE runs 2 big ACTs per step.
  - Input projections folded on host into per-class tables; one-hot tables built on
    host with 4-band replication so the 4 per-gate one-hot matmuls run concurrently
    in disjoint PE row groups (tile_position row tiling).
  - Encoder recurrent matmuls in fp8e4 DoubleRow (K=256 per MM, 8 MMs/step/dir);
    decoder in bf16 (latency-bound, keeps logits/argmax accurate).
  - Gates/cell state in bf16 (DVE 2x mode); per-step h transpose on TensorE into a
    PSUM slot recycled from the gate chunk, then cast to fp8 (enc) / bf16 (dec).
"""

import os
import sys

for _p in ("/opt/trn_rl_repo", "/root/.axon_site/_ro/trn_rl_repo"):
    if os.path.isdir(_p) and _p not in sys.path:
        sys.path.append(_p)

import numpy as np
import ml_dtypes
import concourse.bass as bass
import concourse.tile as tile
from concourse import bacc, mybir
from concourse.bass_utils import run_bass_kernel_spmd

N_CORES = 8
B_LOC = 128
H = 512
G = 2048
VIN = 16
VOUT = 9

f32 = mybir.dt.float32
bf16 = mybir.dt.bfloat16
f8 = mybir.dt.float8e4
i32 = mybir.dt.int32
u32 = mybir.dt.uint32
AF = mybir.ActivationFunctionType
ALU = mybir.AluOpType
AX = mybir.AxisListType
DR = mybir.MatmulPerfMode.DoubleRow

ENC_FP8 = os.environ.get("K_ENC_FP8", "1") == "1"
DEC_FP8 = os.environ.get("K_DEC_FP8", "1") == "1"
ehdt = f8 if ENC_FP8 else bf16

np_bf16 = ml_dtypes.bfloat16
np_f8 = ml_dtypes.float8_e4m3

# device gate order: chunk A = (i, g), chunk B = (f, o); reference order i,f,g,o
GPERM = [0, 2, 1, 3]  # device block k holds reference gate GPERM[k]


def build_program(S=128, T=48):
    nc = bacc.Bacc("TRN2", target_bir_lowering=False, debug=False)

    OHTS = nc.dram_tensor("ohts", [B_LOC, S * 128], bf16, kind="ExternalInput").ap()
    WH8 = {
        "f": nc.dram_tensor("wh8f", [128, 2 * 2 * G], f8, kind="ExternalInput").ap(),
        "b": nc.dram_tensor("wh8b", [128, 2 * 2 * G], f8, kind="ExternalInput").ap(),
        "d": nc.dram_tensor("wh8d", [128, 2 * 2 * G], f8, kind="ExternalInput").ap(),
    }
    WH16 = {
        "f": nc.dram_tensor("wh16f", [128, 4 * G], bf16, kind="ExternalInput").ap(),
        "b": nc.dram_tensor("wh16b", [128, 4 * G], bf16, kind="ExternalInput").ap(),
        "d": nc.dram_tensor("wh16d", [128, 4 * G], bf16, kind="ExternalInput").ap(),
    }
    ZX = {
        "f": nc.dram_tensor("zxf", [128, G], bf16, kind="ExternalInput").ap(),
        "b": nc.dram_tensor("zxb", [128, G], bf16, kind="ExternalInput").ap(),
    }
    ZED = nc.dram_tensor("zed", [128, G], bf16, kind="ExternalInput").ap()
    WOUT = nc.dram_tensor("wout", [128, 64], bf16, kind="ExternalInput").ap()
    BOUT = nc.dram_tensor("bout", [1, 16], bf16, kind="ExternalInput").ap()
    IDENT = nc.dram_tensor("ident", [128, 128], bf16, kind="ExternalInput").ap()
    IOTA9 = nc.dram_tensor("iota9", [128, 128], f32, kind="ExternalInput").ap()
    ONES = nc.dram_tensor("ones", [1, 128], bf16, kind="ExternalInput").ap()
    OH0 = nc.dram_tensor("oh0", [128, 128], bf16, kind="ExternalInput").ap()
    PREDS = nc.dram_tensor("preds", [B_LOC, T * VOUT], f32, kind="ExternalOutput").ap()

    with tile.TileContext(nc) as tc:
        _emit(nc, tc, S, T, OHTS, WH8, WH16, ZX, ZED, WOUT, BOUT, IDENT, IOTA9, ONES, OH0, PREDS)
    nc.compile()
    return nc


def _emit(nc, tc, S, T, OHTS, WH8, WH16, ZX, ZED, WOUT, BOUT, IDENT, IOTA9, ONES, OH0, PREDS):
    from contextlib import ExitStack

    ctx = ExitStack()
    const = ctx.enter_context(tc.tile_pool(name="const", bufs=1))

    # ---------------- phase 0: constants into SBUF (all pre-converted on host) --------
    ohts = const.tile([128, S * 128], bf16, name="ohts_sb")
    CH = 8
    chw = S * 128 // CH
    order = [0, CH - 1, 1, CH - 2, 2, CH - 3, 3, CH - 4][:CH]
    for k in order:
        nc.sync.dma_start(ohts[:, k * chw : (k + 1) * chw], OHTS[:, k * chw : (k + 1) * chw])

    zxr = {}
    for d in ("f", "b"):
        zt = const.tile([128, G], bf16, name=f"zxr{d}")
        nc.sync.dma_start(zt[:], ZX[d][:])
        zxr[d] = zt
    wh8 = {}
    f8dirs = (("f", "b", "d") if DEC_FP8 else ("f", "b")) if ENC_FP8 else ()
    for d in f8dirs:
        wt = const.tile([128, 2 * 2 * G], f8, name=f"wh8{d}")
        nc.sync.dma_start(wt[:], WH8[d][:])
        wh8[d] = wt
    wh16 = {}
    for d in ("f", "b", "d"):
        if d in f8dirs:
            continue
        wt = const.tile([128, 4 * G], bf16, name=f"wh16{d}")
        nc.sync.dma_start(wt[:], WH16[d][:])
        wh16[d] = wt
    zed_sb = const.tile([128, G], bf16, name="zed_sb")
    nc.sync.dma_start(zed_sb[:], ZED[:])
    wout_sb = const.tile([128, 64], bf16, name="wout_sb")
    nc.sync.dma_start(wout_sb[:], WOUT[:])
    bout_sb = const.tile([1, 16], bf16, name="bout_sb")
    nc.sync.dma_start(bout_sb[:], BOUT[:])
    ident_sb = const.tile([128, 128], bf16, name="ident_sb")
    nc.sync.dma_start(ident_sb[:], IDENT[:])
    iota9_sb = const.tile([128, 128], f32, name="iota9_sb")
    nc.sync.dma_start(iota9_sb[:], IOTA9[:])
    ones_sb = const.tile([1, 128], bf16, name="ones_sb")
    nc.sync.dma_start(ones_sb[:], ONES[:])
    oh0_sb = const.tile([128, 128], bf16, name="oh0_sb")
    nc.sync.dma_start(oh0_sb[:], OH0[:])

    # ---------------- PSUM pools: 2 chunks x [128,1024] f32 per direction = 8 banks ---
    pz = ctx.enter_context(tc.tile_pool(name="pz", bufs=1, space="PSUM"))
    # SBUF pools
    pg = ctx.enter_context(tc.tile_pool(name="pg", bufs=2))
    pc = ctx.enter_context(tc.tile_pool(name="pc", bufs=2))
    ph = ctx.enter_context(tc.tile_pool(name="ph", bufs=2))

    def alloc_z(d, t):
        dtag = "f" if d == "d" else d
        zA = pz.tile([128, 1024], f32, tag=f"zA{dtag}", name=f"zA{d}{t}")
        zB = pz.tile([128, 1024], f32, tag=f"zB{dtag}", name=f"zB{d}{t}")
        return zA, zB

    def zslice(zA, zB, k):
        # device gate block k: 0=i (A lo), 1=g (A hi), 2=f (B lo), 3=o (B hi)
        zc = zA if k < 2 else zB
        return zc[:, (k % 2) * 512 : (k % 2) * 512 + 512]

    def enc_onehots(d, t, tok_step, zA, zB, only):
        # 4 concurrent one-hot matmuls, one per gate block, in disjoint row bands
        for k in range(4):
            nc.tensor.matmul(
                zslice(zA, zB, k),
                ohts[32 * k : 32 * k + VIN, tok_step * 128 : (tok_step + 1) * 128],
                zxr[d][32 * k : 32 * k + VIN, k * 512 : (k + 1) * 512],
                start=True,
                stop=only,
                tile_position=(32 * k, 0),
            )

    def enc_mains_fp8(d, t, zA, zB, hT, opener=False):
        for k in range(4):
            dst = zslice(zA, zB, k)
            for i in range(2):
                lhsT = hT[:, 256 * i : 256 * i + 256].rearrange("p (j m) -> p j m", j=2)
                rhs = wh8[d][:, 4096 * i + 2048 * 0 : 4096 * i + 4096].rearrange(
                    "p (j n) -> p j n", j=2
                )[:, :, k * 512 : (k + 1) * 512]
                nc.tensor.matmul(
                    dst, lhsT, rhs, start=(opener and i == 0), stop=(i == 1), perf_mode=DR
                )

    def mains_bf16(d, t, zA, zB, hT, wtile, opener=False):
        for k in range(4):
            dst = zslice(zA, zB, k)
            for kc in range(4):
                nc.tensor.matmul(
                    dst,
                    hT[:, kc * 128 : (kc + 1) * 128],
                    wtile[:, kc * G + k * 512 : kc * G + (k + 1) * 512],
                    start=(opener and kc == 0),
                    stop=(kc == 3),
                )

    def sig(d, t, zA, zB):
        """Two sigmoid chunks: gA = sigma(i,g pre-acts), gB = sigma(f,o)."""
        dtag = "f" if d == "d" else d
        gA = pg.tile([128, 1024], bf16, tag=f"gA{dtag}", name=f"gA{d}{t}")
        gB = pg.tile([128, 1024], bf16, tag=f"gB{dtag}", name=f"gB{d}{t}")
        nc.scalar.activation(gA[:], zA[:], AF.Sigmoid)
        nc.scalar.activation(gB[:], zB[:], AF.Sigmoid)
        return gA, gB

    def cell(d, t, gA, gB, c_prev, first):
        """DVE cell update -> c2 (bf16)."""
        dtag = "f" if d == "d" else d
        g2 = pg.tile([128, 512], bf16, tag=f"g2{dtag}", name=f"g2{d}{t}")
        nc.vector.tensor_scalar(g2[:], gA[:, 512:1024], 2.0, 1.0, ALU.mult, ALU.subtract)
        c_new = pc.tile([128, 512], bf16, tag=f"c{dtag}", name=f"c{d}{t}")
        if first:
            nc.vector.tensor_tensor(c_new[:], gA[:, 0:512], g2[:], ALU.mult)
        else:
            t1 = pg.tile([128, 512], bf16, tag=f"t1{dtag}", name=f"t1{d}{t}")
            nc.vector.tensor_tensor(t1[:], gA[:, 0:512], g2[:], ALU.mult)
            nc.vector.tensor_tensor(c_new[:], gB[:, 0:512], c_prev[:], ALU.mult)
            nc.vector.tensor_tensor(c_new[:], c_new[:], t1[:], ALU.add)
        return c_new

    def tr_o(d, t, gB):
        """Transpose the o gate into the zB PSUM slot (free after sigma read it)."""
        dtag = "f" if d == "d" else d
        trPo = pz.tile([128, 512], bf16, tag=f"zB{dtag}", name=f"tro{d}{t}")
        for kc in range(4):
            nc.tensor.transpose(
                trPo[:, kc * 128 : (kc + 1) * 128],
                gB[:, 512 + kc * 128 : 512 + (kc + 1) * 128],
                ident_sb[:],
            )
        return trPo

    def tr_c(d, t, c_new):
        dtag = "f" if d == "d" else d
        trPc = pz.tile([128, 512], bf16, tag=f"zA{dtag}", name=f"trc{d}{t}")
        for kc in range(4):
            nc.tensor.transpose(
                trPc[:, kc * 128 : (kc + 1) * 128], c_new[:, kc * 128 : (kc + 1) * 128], ident_sb[:]
            )
        return trPc

    def fin(d, t, trPo, trPc, out_dt):
        """tanh on transposed cell, then hT = oT * tanh(cT) directly in hT layout."""
        dtag = "f" if d == "d" else d
        tcT = pg.tile([128, 512], bf16, tag=f"tc{dtag}", name=f"tc{d}{t}")
        nc.scalar.activation(tcT[:], trPc[:], AF.Tanh)
        hT_new = ph.tile([128, 512], out_dt, tag=f"hT{dtag}{out_dt}", name=f"hT{d}{t}")
        nc.vector.tensor_tensor(hT_new[:], trPo[:], tcT[:], ALU.mult)
        return hT_new

    # ---------------- encoder ----------------------------------------------------------
    cs = {"f": None, "b": None}
    hTs = {"f": None, "b": None}
    for t in range(S):
        zf = alloc_z("f", t)
        enc_onehots("f", t, t, *zf, only=(t == 0))
        if t > 0:
            if ENC_FP8:
                enc_mains_fp8("f", t, *zf, hTs["f"])
            else:
                mains_bf16("f", t, *zf, hTs["f"], wh16["f"])
        zb = alloc_z("b", t)
        enc_onehots("b", t, S - 1 - t, *zb, only=(t == 0))
        if t > 0:
            if ENC_FP8:
                enc_mains_fp8("b", t, *zb, hTs["b"])
            else:
                mains_bf16("b", t, *zb, hTs["b"], wh16["b"])
        # breadth-first tails so f/b alternate in each engine's FIFO
        gAf, gBf = sig("f", t, *zf)
        gAb, gBb = sig("b", t, *zb)
        cs["f"] = cell("f", t, gAf, gBf, cs["f"], t == 0)
        cs["b"] = cell("b", t, gAb, gBb, cs["b"], t == 0)
        trPo_f = tr_o("f", t, gBf)
        trPc_f = tr_c("f", t, cs["f"])
        trPo_b = tr_o("b", t, gBb)
        trPc_b = tr_c("b", t, cs["b"])
        hTs["f"] = fin("f", t, trPo_f, trPc_f, ehdt if t < S - 1 else bf16)
        hTs["b"] = fin("b", t, trPo_b, trPc_b, ehdt if t < S - 1 else bf16)

    # decoder init: sum of final fwd/bwd states (both bf16 casts on the last step)
    dhdt = f8 if (ENC_FP8 and DEC_FP8) else bf16
    c_d = pc.tile([128, 512], bf16, tag="cf", name="cd_init")
    nc.vector.tensor_tensor(c_d[:], cs["f"][:], cs["b"][:], ALU.add)
    hT_d = ph.tile([128, 512], dhdt, tag=f"hTf{dhdt}", name="hTd_init")
    nc.vector.tensor_tensor(hT_d[:], hTs["f"][:], hTs["b"][:], ALU.add)

    # ---------------- decoder ----------------------------------------------------------
    logits_all = const.tile([B_LOC, T * VOUT], f32, name="logits_all")
    psm = ctx.enter_context(tc.tile_pool(name="psm", bufs=2))

    ohT4 = oh0_sb
    prev_ohp = None
    for t in range(T):
        zA, zB = alloc_z("d", t)
        # mains first so the PE isn't head-of-line blocked on the argmax chain
        if ENC_FP8 and DEC_FP8:
            enc_mains_fp8("d", t, zA, zB, hT_d, opener=True)
        else:
            mains_bf16("d", t, zA, zB, hT_d, wh16["d"], opener=True)
        junk = pz.tile([128, 512], bf16, tag="zAb", name=f"junk{t}")
        if prev_ohp is not None:
            nc.tensor.transpose(junk[:, 0:128], prev_ohp[:], ident_sb[:])
        for k in range(4):
            nc.tensor.matmul(
                zslice(zA, zB, k),
                ohT4[32 * k : 32 * k + VOUT, :],
                zed_sb[32 * k : 32 * k + VOUT, k * 512 : (k + 1) * 512],
                start=False,
                stop=True,
                tile_position=(32 * k, 0),
            )
        gA, gB = sig("d", t, zA, zB)
        # HAM-warming work with staggered real deps (outputs unused): keeps the
        # PE busy through the serial argmax/cell chain so it stays at 2.4 GHz
        nc.tensor.transpose(junk[:, 128:256], gA[:, 0:128], ident_sb[:])
        c_d = cell("d", t, gA, gB, c_d, False)
        nc.tensor.transpose(junk[:, 256:384], gB[:, 0:128], ident_sb[:])
        trPo = tr_o("d", t, gB)
        trPc = tr_c("d", t, c_d)
        nc.tensor.transpose(junk[:, 384:512], c_d[:, 0:128], ident_sb[:])
        hT_d = fin("d", t, trPo, trPc, dhdt)

        lgP = pz.tile([128, 16], f32, tag="zAb", name=f"lg{t}")
        nc.tensor.matmul(lgP[:], ones_sb[0:1, :], bout_sb[0:1, :], start=True, stop=False)
        for kc in range(4):
            nc.tensor.matmul(
                lgP[:],
                hT_d[:, kc * 128 : (kc + 1) * 128],
                wout_sb[:, kc * 16 : (kc + 1) * 16],
                start=False,
                stop=(kc == 3),
            )
        lg_sb = logits_all[:, t * VOUT : (t + 1) * VOUT]
        nc.vector.tensor_copy(lg_sb, lgP[:, 0:VOUT])

        if t < T - 1:
            lmax = psm.tile([128, 8], f32, tag="lmax", name=f"lmax{t}")
            nc.vector.max(lmax[:], lg_sb)
            yidx = psm.tile([128, 8], u32, tag="yidx", name=f"yidx{t}")
            nc.vector.max_index(yidx[:], lmax[:], lg_sb)
            yf = psm.tile([128, 1], f32, tag="yf", name=f"yf{t}")
            nc.vector.tensor_copy(yf[:], yidx[:, 0:1])
            # 4-band one-hot in [v, b] layout via one PE transpose
            ohp4 = psm.tile([128, 128], bf16, tag="ohp", name=f"ohp{t}")
            nc.vector.tensor_scalar(ohp4[:], iota9_sb[:], yf[:, 0:1], None, ALU.is_equal)
            prev_ohp = ohp4
            trOH = pz.tile([128, 128], bf16, tag="zBb", name=f"troh{t}")
            nc.tensor.transpose(trOH[:], ohp4[:], ident_sb[:])
            ohT4 = psm.tile([128, 128], bf16, tag="ohT", name=f"ohT{t}")
            nc.vector.tensor_copy(ohT4[:], trOH[:])

    # ---------------- batched softmax over all logits -----------------------------------
    preds_sb = const.tile([B_LOC, T * VOUT], f32, name="preds_sb")
    exps = const.tile([B_LOC, T * VOUT], f32, name="exps")
    lmax48 = const.tile([128, T], f32, name="lmax48")
    lview = logits_all[:].rearrange("p (t v) -> p t v", v=VOUT)
    nc.vector.tensor_reduce(lmax48[:], lview, AX.X, ALU.max)
    for t in range(T):
        nc.vector.tensor_scalar(
            exps[:, t * VOUT : (t + 1) * VOUT],
            logits_all[:, t * VOUT : (t + 1) * VOUT],
            lmax48[:, t : t + 1],
            None,
            ALU.subtract,
        )
    nc.scalar.activation(exps[:], exps[:], AF.Exp)
    sums48 = const.tile([128, T], f32, name="sums48")
    eview = exps[:].rearrange("p (t v) -> p t v", v=VOUT)
    nc.vector.tensor_reduce(sums48[:], eview, AX.X, ALU.add)
    rec48 = const.tile([128, T], f32, name="rec48")
    nc.vector.reciprocal(rec48[:], sums48[:])
    for t in range(T):
        nc.vector.tensor_scalar(
            preds_sb[:, t * VOUT : (t + 1) * VOUT],
            exps[:, t * VOUT : (t + 1) * VOUT],
            rec48[:, t : t + 1],
            None,
            ALU.mult,
        )
    nc.sync.dma_start(PREDS[:], preds_sb[:])
    ctx.close()


_PROGRAM_CACHE = {}


def _get_program(S=128, T=48):
    key = (S, T)
    if key not in _PROGRAM_CACHE:
        _PROGRAM_CACHE[key] = build_program(S, T)
    return _PROGRAM_CACHE[key]


def _perm_scale(W):
    """Reorder gate blocks (i,f,g,o)->(i,g,f,o) and scale the g block by 2."""
    Hd = W.shape[0]
    W4 = np.asarray(W, np.float64).reshape(Hd, 4, 512)
    out = np.empty_like(W4)
    out[:, 0] = W4[:, 0]
    out[:, 1] = 2.0 * W4[:, 2]
    out[:, 2] = W4[:, 1]
    out[:, 3] = W4[:, 3]
    return out.reshape(Hd, 2048)


def _wh8_arrange(W):
    # W: [512, 2048] f64 -> [128, 2*2*2048] fp8 with value at col (i*4096+j*2048+n)
    # = W[256i+128j+ki, n]
    W4 = W.reshape(2, 2, 128, 2048)  # [i, j, ki, n]
    arr = np.ascontiguousarray(np.transpose(W4, (2, 0, 1, 3)).reshape(128, 8192))
    return arr.astype(np_f8)


def _wh16_arrange(W):
    # W: [512, 2048] -> [128, 4*2048] bf16 with value at col (kc*2048+n) = W[kc*128+p, n]
    return np.ascontiguousarray(W.reshape(4, 128, 2048).transpose(1, 0, 2).reshape(128, 8192)).astype(np_bf16)


def _zx_rep(zx):
    # [16, 2048] -> [128, 2048] bf16 with the table replicated at rows 32j+v
    out = np.zeros((128, 2048), np.float64)
    for j in range(4):
        out[32 * j : 32 * j + VIN] = zx
    return out.astype(np_bf16)


def _onehot_table(tokens, S):
    # [128, S*128] bf16: row 32j+v, col t*128+b = (tokens[b, t] == v)
    oh = (np.arange(VIN)[:, None, None] == tokens.T[None, :, :])  # [16, S, 128b]
    arr = np.zeros((32, S, 128), np.float32)
    arr[:VIN] = oh
    flat = arr.reshape(32, S * 128)
    return np.tile(flat, (4, 1)).astype(np_bf16)


def make_in_maps(tokens, Wh_f, Wh_b, Wh_d, zx_f, zx_b, ze_d, W_out, b_out):
    B = tokens.shape[0]
    assert B % N_CORES == 0
    bl = B // N_CORES
    ident = np.eye(128, dtype=np.float32).astype(np_bf16)
    iota9 = np.tile(np.tile(np.arange(32, dtype=np.float32), 4), (128, 1))
    ones = np.ones((1, 128), np.float32).astype(np_bf16)
    oh0 = np.zeros((128, 128), np.float32)
    oh0[::32, :] = 1.0
    oh0 = oh0.astype(np_bf16)

    whf = _perm_scale(Wh_f)
    whb = _perm_scale(Wh_b)
    whd = _perm_scale(Wh_d)
    zedp = np.zeros((128, 2048), np.float64)
    zep = _perm_scale(ze_d)
    for j in range(4):
        zedp[32 * j : 32 * j + VOUT] = zep

    wout = np.zeros((128, 64), np.float32)
    Wov = np.asarray(W_out, np.float32).reshape(4, 128, VOUT)
    for kc in range(4):
        wout[:, kc * 16 : kc * 16 + VOUT] = Wov[kc]

    common = dict(
        wh16d=_wh16_arrange(whd),
        zxf=_zx_rep(_perm_scale(zx_f)[:VIN]),
        zxb=_zx_rep(_perm_scale(zx_b)[:VIN]),
        zed=zedp.astype(np_bf16),
        wout=wout.astype(np_bf16),
        bout=np.pad(np.asarray(b_out, np.float32).reshape(1, VOUT), ((0, 0), (0, 16 - VOUT))).astype(np_bf16),
        ident=ident, iota9=iota9, ones=ones, oh0=oh0,
    )
    # the program declares all weight tensors; feed them all (unused ones are
    # never DMA'd by the device program)
    common["wh8f"] = _wh8_arrange(whf)
    common["wh8b"] = _wh8_arrange(whb)
    common["wh8d"] = _wh8_arrange(whd)
    common["wh16f"] = _wh16_arrange(whf)
    common["wh16b"] = _wh16_arrange(whb)
    S = tokens.shape[1]
    return [
        {**common, "ohts": _onehot_table(tokens[c * bl : (c + 1) * bl], S)}
        for c in range(N_CORES)
    ]


def fold_tables(emb_in, Wi_f, b_f, Wi_b, b_b, emb_out, Wi_d, b_d):
    f8_ = lambda x: np.asarray(x, np.float64)
    zx_f = f8_(emb_in) @ f8_(Wi_f) + f8_(b_f)
    zx_b = f8_(emb_in) @ f8_(Wi_b) + f8_(b_b)
    ze_d = f8_(emb_out) @ f8_(Wi_d) + f8_(b_d)
    return zx_f, zx_b, ze_d


def kernel(tokens, emb_in, Wi_f, Wh_f, b_f, Wi_b, Wh_b, b_b,
           emb_out, Wi_d, Wh_d, b_d, W_out, b_out, max_length):
    T = int(max_length)
    tokens = np.asarray(tokens, np.int32)
    B, S = tokens.shape
    zx_f, zx_b, ze_d = fold_tables(emb_in, Wi_f, b_f, Wi_b, b_b, emb_out, Wi_d, b_d)
    nc = _get_program(S, T)
    in_maps = make_in_maps(tokens, Wh_f, Wh_b, Wh_d, zx_f, zx_b, ze_d, W_out, b_out)
    res = run_bass_kernel_spmd(nc, in_maps, list(range(N_CORES)))
    bl = B // N_CORES
    preds = np.concatenate(
        [res.results[c]["preds"].reshape(bl, T, VOUT) for c in range(N_CORES)], axis=0
    )
    return np.ascontiguousarray(preds, np.float32)


# revision 13
# speedup vs baseline: 1.2432x; 1.0185x over previous
"""Trainium2 Bass kernel for nn_AbsSeq2SeqLSTM (bi-LSTM encoder + greedy-argmax LSTM decoder).

Sharding: pure data parallelism - batch 1024 split as 128 per NeuronCore across 8 cores;
all weights replicated.

Key structure (per core, B_local=128, S=128, H=512, T=48):
  - All four gates through ONE sigmoid lookup: tanh(x) = 2*sigmoid(2x)-1, with the
    g-gate's weight columns pre-scaled by 2 on host. Gate blocks laid out (i,g | f,o)
    in two [128,1024] PSUM chunks per direction so ScalarE runs 2 big ACTs per step.
  - Input projections folded on host into per-class tables; one-hot tables built on
    host with 4-band replication so the 4 per-gate one-hot matmuls run concurrently
    in disjoint PE row groups (tile_position row tiling).
  - Encoder recurrent matmuls in fp8e4 DoubleRow (K=256 per MM, 8 MMs/step/dir);
    decoder in bf16 (latency-bound, keeps logits/argmax accurate).
  - Gates/cell state in bf16 (DVE 2x mode); per-step h transpose on TensorE into a
    PSUM slot recycled from the gate chunk, then cast to fp8 (enc) / bf16 (dec).
"""

import os
import sys

for _p in ("/opt/trn_rl_repo", "/root/.axon_site/_ro/trn_rl_repo"):
    if os.path.isdir(_p) and _p not in sys.path:
        sys.path.append(_p)

import numpy as np
import ml_dtypes
import concourse.bass as bass
import concourse.tile as tile
from concourse import bacc, mybir
from concourse.bass_utils import run_bass_kernel_spmd

N_CORES = 8
B_LOC = 128
H = 512
G = 2048
VIN = 16
VOUT = 9

f32 = mybir.dt.float32
bf16 = mybir.dt.bfloat16
f8 = mybir.dt.float8e4
i32 = mybir.dt.int32
u32 = mybir.dt.uint32
AF = mybir.ActivationFunctionType
ALU = mybir.AluOpType
AX = mybir.AxisListType
DR = mybir.MatmulPerfMode.DoubleRow

ENC_FP8 = os.environ.get("K_ENC_FP8", "1") == "1"
DEC_FP8 = os.environ.get("K_DEC_FP8", "1") == "1"
ehdt = f8 if ENC_FP8 else bf16

np_bf16 = ml_dtypes.bfloat16
np_f8 = ml_dtypes.float8_e4m3

# device gate order: chunk A = (i, g), chunk B = (f, o); reference order i,f,g,o
GPERM = [0, 2, 1, 3]  # device block k holds reference gate GPERM[k]


def build_program(S=128, T=48):
    nc = bacc.Bacc("TRN2", target_bir_lowering=False, debug=False)

    OHTS = nc.dram_tensor("ohts", [B_LOC, S * 128], bf16, kind="ExternalInput").ap()
    WH8 = {
        "f": nc.dram_tensor("wh8f", [128, 2 * 2 * G], f8, kind="ExternalInput").ap(),
        "b": nc.dram_tensor("wh8b", [128, 2 * 2 * G], f8, kind="ExternalInput").ap(),
        "d": nc.dram_tensor("wh8d", [128, 2 * 2 * G], f8, kind="ExternalInput").ap(),
    }
    WH16 = {
        "f": nc.dram_tensor("wh16f", [128, 4 * G], bf16, kind="ExternalInput").ap(),
        "b": nc.dram_tensor("wh16b", [128, 4 * G], bf16, kind="ExternalInput").ap(),
        "d": nc.dram_tensor("wh16d", [128, 4 * G], bf16, kind="ExternalInput").ap(),
    }
    ZX = {
        "f": nc.dram_tensor("zxf", [128, G], bf16, kind="ExternalInput").ap(),
        "b": nc.dram_tensor("zxb", [128, G], bf16, kind="ExternalInput").ap(),
    }
    ZED = nc.dram_tensor("zed", [128, G], bf16, kind="ExternalInput").ap()
    WOUT = nc.dram_tensor("wout", [128, 64], bf16, kind="ExternalInput").ap()
    BOUT = nc.dram_tensor("bout", [1, 16], bf16, kind="ExternalInput").ap()
    IDENT = nc.dram_tensor("ident", [128, 128], bf16, kind="ExternalInput").ap()
    IOTA9 = nc.dram_tensor("iota9", [128, 128], f32, kind="ExternalInput").ap()
    ONES = nc.dram_tensor("ones", [1, 128], bf16, kind="ExternalInput").ap()
    OH0 = nc.dram_tensor("oh0", [128, 128], bf16, kind="ExternalInput").ap()
    PREDS = nc.dram_tensor("preds", [B_LOC, T * VOUT], f32, kind="ExternalOutput").ap()

    with tile.TileContext(nc) as tc:
        _emit(nc, tc, S, T, OHTS, WH8, WH16, ZX, ZED, WOUT, BOUT, IDENT, IOTA9, ONES, OH0, PREDS)
    nc.compile()
    return nc


def _emit(nc, tc, S, T, OHTS, WH8, WH16, ZX, ZED, WOUT, BOUT, IDENT, IOTA9, ONES, OH0, PREDS):
    from contextlib import ExitStack

    ctx = ExitStack()
    const = ctx.enter_context(tc.tile_pool(name="const", bufs=1))

    # ---------------- phase 0: constants into SBUF (all pre-converted on host) --------
    ohts = const.tile([128, S * 128], bf16, name="ohts_sb")
    CH = 8
    chw = S * 128 // CH
    order = [0, CH - 1, 1, CH - 2, 2, CH - 3, 3, CH - 4][:CH]
    for k in order:
        nc.sync.dma_start(ohts[:, k * chw : (k + 1) * chw], OHTS[:, k * chw : (k + 1) * chw])

    zxr = {}
    for d in ("f", "b"):
        zt = const.tile([128, G], bf16, name=f"zxr{d}")
        nc.sync.dma_start(zt[:], ZX[d][:])
        zxr[d] = zt
    wh8 = {}
    f8dirs = (("f", "b", "d") if DEC_FP8 else ("f", "b")) if ENC_FP8 else ()
    for d in f8dirs:
        wt = const.tile([128, 2 * 2 * G], f8, name=f"wh8{d}")
        nc.sync.dma_start(wt[:], WH8[d][:])
        wh8[d] = wt
    wh16 = {}
    for d in ("f", "b", "d"):
        if d in f8dirs:
            continue
        wt = const.tile([128, 4 * G], bf16, name=f"wh16{d}")
        nc.sync.dma_start(wt[:], WH16[d][:])
        wh16[d] = wt
    zed_sb = const.tile([128, G], bf16, name="zed_sb")
    nc.sync.dma_start(zed_sb[:], ZED[:])
    wout_sb = const.tile([128, 64], bf16, name="wout_sb")
    nc.sync.dma_start(wout_sb[:], WOUT[:])
    bout_sb = const.tile([1, 16], bf16, name="bout_sb")
    nc.sync.dma_start(bout_sb[:], BOUT[:])
    ident_sb = const.tile([128, 128], bf16, name="ident_sb")
    nc.sync.dma_start(ident_sb[:], IDENT[:])
    iota9_sb = const.tile([128, 128], f32, name="iota9_sb")
    nc.sync.dma_start(iota9_sb[:], IOTA9[:])
    ones_sb = const.tile([1, 128], bf16, name="ones_sb")
    nc.sync.dma_start(ones_sb[:], ONES[:])
    oh0_sb = const.tile([128, 128], bf16, name="oh0_sb")
    nc.sync.dma_start(oh0_sb[:], OH0[:])

    # ---------------- PSUM pools: 2 chunks x [128,1024] f32 per direction = 8 banks ---
    pz = ctx.enter_context(tc.tile_pool(name="pz", bufs=1, space="PSUM"))
    # SBUF pools
    pg = ctx.enter_context(tc.tile_pool(name="pg", bufs=2))
    pc = ctx.enter_context(tc.tile_pool(name="pc", bufs=2))
    ph = ctx.enter_context(tc.tile_pool(name="ph", bufs=2))

    def alloc_z(d, t):
        dtag = "f" if d == "d" else d
        zA = pz.tile([128, 1024], f32, tag=f"zA{dtag}", name=f"zA{d}{t}")
        zB = pz.tile([128, 1024], f32, tag=f"zB{dtag}", name=f"zB{d}{t}")
        return zA, zB

    def zslice(zA, zB, k):
        # device gate block k: 0=i (A lo), 1=g (A hi), 2=f (B lo), 3=o (B hi)
        zc = zA if k < 2 else zB
        return zc[:, (k % 2) * 512 : (k % 2) * 512 + 512]

    def enc_onehots(d, t, tok_step, zA, zB, only):
        # 4 concurrent one-hot matmuls, one per gate block, in disjoint row bands
        for k in range(4):
            nc.tensor.matmul(
                zslice(zA, zB, k),
                ohts[32 * k : 32 * k + VIN, tok_step * 128 : (tok_step + 1) * 128],
                zxr[d][32 * k : 32 * k + VIN, k * 512 : (k + 1) * 512],
                start=True,
                stop=only,
                tile_position=(32 * k, 0),
            )

    def enc_mains_fp8(d, t, zA, zB, hT, opener=False):
        for k in range(4):
            dst = zslice(zA, zB, k)
            for i in range(2):
                lhsT = hT[:, 256 * i : 256 * i + 256].rearrange("p (j m) -> p j m", j=2)
                rhs = wh8[d][:, 4096 * i + 2048 * 0 : 4096 * i + 4096].rearrange(
                    "p (j n) -> p j n", j=2
                )[:, :, k * 512 : (k + 1) * 512]
                nc.tensor.matmul(
                    dst, lhsT, rhs, start=(opener and i == 0), stop=(i == 1), perf_mode=DR
                )

    def mains_bf16(d, t, zA, zB, hT, wtile, opener=False):
        for k in range(4):
            dst = zslice(zA, zB, k)
            for kc in range(4):
                nc.tensor.matmul(
                    dst,
                    hT[:, kc * 128 : (kc + 1) * 128],
                    wtile[:, kc * G + k * 512 : kc * G + (k + 1) * 512],
                    start=(opener and kc == 0),
                    stop=(kc == 3),
                )

    def sig(d, t, zA, zB):
        """Two sigmoid chunks: gA = sigma(i,g pre-acts), gB = sigma(f,o)."""
        dtag = "f" if d == "d" else d
        gA = pg.tile([128, 1024], bf16, tag=f"gA{dtag}", name=f"gA{d}{t}")
        gB = pg.tile([128, 1024], bf16, tag=f"gB{dtag}", name=f"gB{d}{t}")
        nc.scalar.activation(gA[:], zA[:], AF.Sigmoid)
        nc.scalar.activation(gB[:], zB[:], AF.Sigmoid)
        return gA, gB

    def cell(d, t, gA, gB, c_prev, first):
        """DVE cell update -> c2 (bf16)."""
        dtag = "f" if d == "d" else d
        g2 = pg.tile([128, 512], bf16, tag=f"g2{dtag}", name=f"g2{d}{t}")
        nc.vector.tensor_scalar(g2[:], gA[:, 512:1024], 2.0, 1.0, ALU.mult, ALU.subtract)
        c_new = pc.tile([128, 512], bf16, tag=f"c{dtag}", name=f"c{d}{t}")
        if first:
            nc.vector.tensor_tensor(c_new[:], gA[:, 0:512], g2[:], ALU.mult)
        else:
            t1 = pg.tile([128, 512], bf16, tag=f"t1{dtag}", name=f"t1{d}{t}")
            nc.vector.tensor_tensor(t1[:], gA[:, 0:512], g2[:], ALU.mult)
            nc.vector.tensor_tensor(c_new[:], gB[:, 0:512], c_prev[:], ALU.mult)
            nc.vector.tensor_tensor(c_new[:], c_new[:], t1[:], ALU.add)
        return c_new

    def tr_o(d, t, gB):
        """Transpose the o gate into the zB PSUM slot (free after sigma read it)."""
        dtag = "f" if d == "d" else d
        trPo = pz.tile([128, 512], bf16, tag=f"zB{dtag}", name=f"tro{d}{t}")
        for kc in range(4):
            nc.tensor.transpose(
                trPo[:, kc * 128 : (kc + 1) * 128],
                gB[:, 512 + kc * 128 : 512 + (kc + 1) * 128],
                ident_sb[:],
            )
        return trPo

    def tr_c(d, t, c_new):
        dtag = "f" if d == "d" else d
        trPc = pz.tile([128, 512], bf16, tag=f"zA{dtag}", name=f"trc{d}{t}")
        for kc in range(4):
            nc.tensor.transpose(
                trPc[:, kc * 128 : (kc + 1) * 128], c_new[:, kc * 128 : (kc + 1) * 128], ident_sb[:]
            )
        return trPc

    def fin(d, t, trPo, trPc, out_dt):
        """tanh on transposed cell, then hT = oT * tanh(cT) directly in hT layout.
        Split into K-halves: the next step's first DoubleRow matmul needs only
        hT[:, 0:256], so half 0 unblocks it early (subtile deps)."""
        dtag = "f" if d == "d" else d
        tcT = pg.tile([128, 512], bf16, tag=f"tc{dtag}", name=f"tc{d}{t}")
        hT_new = ph.tile([128, 512], out_dt, tag=f"hT{dtag}{out_dt}", name=f"hT{d}{t}")
        for h in range(2):
            cl = slice(h * 256, (h + 1) * 256)
            nc.scalar.activation(tcT[:, cl], trPc[:, cl], AF.Tanh)
            nc.vector.tensor_tensor(hT_new[:, cl], trPo[:, cl], tcT[:, cl], ALU.mult)
        return hT_new

    # ---------------- encoder ----------------------------------------------------------
    cs = {"f": None, "b": None}
    hTs = {"f": None, "b": None}
    for t in range(S):
        zf = alloc_z("f", t)
        enc_onehots("f", t, t, *zf, only=(t == 0))
        if t > 0:
            if ENC_FP8:
                enc_mains_fp8("f", t, *zf, hTs["f"])
            else:
                mains_bf16("f", t, *zf, hTs["f"], wh16["f"])
        zb = alloc_z("b", t)
        enc_onehots("b", t, S - 1 - t, *zb, only=(t == 0))
        if t > 0:
            if ENC_FP8:
                enc_mains_fp8("b", t, *zb, hTs["b"])
            else:
                mains_bf16("b", t, *zb, hTs["b"], wh16["b"])
        # breadth-first tails so f/b alternate in each engine's FIFO
        gAf, gBf = sig("f", t, *zf)
        gAb, gBb = sig("b", t, *zb)
        cs["f"] = cell("f", t, gAf, gBf, cs["f"], t == 0)
        cs["b"] = cell("b", t, gAb, gBb, cs["b"], t == 0)
        trPo_f = tr_o("f", t, gBf)
        trPc_f = tr_c("f", t, cs["f"])
        trPo_b = tr_o("b", t, gBb)
        trPc_b = tr_c("b", t, cs["b"])
        hTs["f"] = fin("f", t, trPo_f, trPc_f, ehdt if t < S - 1 else bf16)
        hTs["b"] = fin("b", t, trPo_b, trPc_b, ehdt if t < S - 1 else bf16)

    # decoder init: sum of final fwd/bwd states (both bf16 casts on the last step)
    dhdt = f8 if (ENC_FP8 and DEC_FP8) else bf16
    c_d = pc.tile([128, 512], bf16, tag="cf", name="cd_init")
    nc.vector.tensor_tensor(c_d[:], cs["f"][:], cs["b"][:], ALU.add)
    hT_d = ph.tile([128, 512], dhdt, tag=f"hTf{dhdt}", name="hTd_init")
    nc.vector.tensor_tensor(hT_d[:], hTs["f"][:], hTs["b"][:], ALU.add)

    # ---------------- decoder ----------------------------------------------------------
    logits_all = const.tile([B_LOC, T * VOUT], f32, name="logits_all")
    psm = ctx.enter_context(tc.tile_pool(name="psm", bufs=2))

    ohT4 = oh0_sb
    prev_ohp = None
    for t in range(T):
        zA, zB = alloc_z("d", t)
        # mains first so the PE isn't head-of-line blocked on the argmax chain
        if ENC_FP8 and DEC_FP8:
            enc_mains_fp8("d", t, zA, zB, hT_d, opener=True)
        else:
            mains_bf16("d", t, zA, zB, hT_d, wh16["d"], opener=True)
        junk = pz.tile([128, 512], bf16, tag="zAb", name=f"junk{t}")
        if prev_ohp is not None:
            nc.tensor.transpose(junk[:, 0:128], prev_ohp[:], ident_sb[:])
        for k in range(4):
            nc.tensor.matmul(
                zslice(zA, zB, k),
                ohT4[32 * k : 32 * k + VOUT, :],
                zed_sb[32 * k : 32 * k + VOUT, k * 512 : (k + 1) * 512],
                start=False,
                stop=True,
                tile_position=(32 * k, 0),
            )
        gA, gB = sig("d", t, zA, zB)
        # HAM-warming work with staggered real deps (outputs unused): keeps the
        # PE busy through the serial argmax/cell chain so it stays at 2.4 GHz
        nc.tensor.transpose(junk[:, 128:256], gA[:, 0:128], ident_sb[:])
        c_d = cell("d", t, gA, gB, c_d, False)
        nc.tensor.transpose(junk[:, 256:384], gB[:, 0:128], ident_sb[:])
        trPo = tr_o("d", t, gB)
        trPc = tr_c("d", t, c_d)
        nc.tensor.transpose(junk[:, 384:512], c_d[:, 0:128], ident_sb[:])
        hT_d = fin("d", t, trPo, trPc, dhdt)

        lgP = pz.tile([128, 16], f32, tag="zAb", name=f"lg{t}")
        nc.tensor.matmul(lgP[:], ones_sb[0:1, :], bout_sb[0:1, :], start=True, stop=False)
        for kc in range(4):
            nc.tensor.matmul(
                lgP[:],
                hT_d[:, kc * 128 : (kc + 1) * 128],
                wout_sb[:, kc * 16 : (kc + 1) * 16],
                start=False,
                stop=(kc == 3),
            )
        lg_sb = logits_all[:, t * VOUT : (t + 1) * VOUT]
        nc.vector.tensor_copy(lg_sb, lgP[:, 0:VOUT])

        if t < T - 1:
            lmax = psm.tile([128, 8], f32, tag="lmax", name=f"lmax{t}")
            nc.vector.max(lmax[:], lg_sb)
            yidx = psm.tile([128, 8], u32, tag="yidx", name=f"yidx{t}")
            nc.vector.max_index(yidx[:], lmax[:], lg_sb)
            yf = psm.tile([128, 1], f32, tag="yf", name=f"yf{t}")
            nc.vector.tensor_copy(yf[:], yidx[:, 0:1])
            # 4-band one-hot in [v, b] layout via one PE transpose
            ohp4 = psm.tile([128, 128], bf16, tag="ohp", name=f"ohp{t}")
            nc.vector.tensor_scalar(ohp4[:], iota9_sb[:], yf[:, 0:1], None, ALU.is_equal)
            prev_ohp = ohp4
            trOH = pz.tile([128, 128], bf16, tag="zBb", name=f"troh{t}")
            nc.tensor.transpose(trOH[:], ohp4[:], ident_sb[:])
            ohT4 = psm.tile([128, 128], bf16, tag="ohT", name=f"ohT{t}")
            nc.vector.tensor_copy(ohT4[:], trOH[:])

    # ---------------- batched softmax over all logits -----------------------------------
    preds_sb = const.tile([B_LOC, T * VOUT], f32, name="preds_sb")
    exps = const.tile([B_LOC, T * VOUT], f32, name="exps")
    lmax48 = const.tile([128, T], f32, name="lmax48")
    lview = logits_all[:].rearrange("p (t v) -> p t v", v=VOUT)
    nc.vector.tensor_reduce(lmax48[:], lview, AX.X, ALU.max)
    for t in range(T):
        nc.vector.tensor_scalar(
            exps[:, t * VOUT : (t + 1) * VOUT],
            logits_all[:, t * VOUT : (t + 1) * VOUT],
            lmax48[:, t : t + 1],
            None,
            ALU.subtract,
        )
    nc.scalar.activation(exps[:], exps[:], AF.Exp)
    sums48 = const.tile([128, T], f32, name="sums48")
    eview = exps[:].rearrange("p (t v) -> p t v", v=VOUT)
    nc.vector.tensor_reduce(sums48[:], eview, AX.X, ALU.add)
    rec48 = const.tile([128, T], f32, name="rec48")
    nc.vector.reciprocal(rec48[:], sums48[:])
    for t in range(T):
        nc.vector.tensor_scalar(
            preds_sb[:, t * VOUT : (t + 1) * VOUT],
            exps[:, t * VOUT : (t + 1) * VOUT],
            rec48[:, t : t + 1],
            None,
            ALU.mult,
        )
    nc.sync.dma_start(PREDS[:], preds_sb[:])
    ctx.close()


_PROGRAM_CACHE = {}


def _get_program(S=128, T=48):
    key = (S, T)
    if key not in _PROGRAM_CACHE:
        _PROGRAM_CACHE[key] = build_program(S, T)
    return _PROGRAM_CACHE[key]


def _perm_scale(W):
    """Reorder gate blocks (i,f,g,o)->(i,g,f,o) and scale the g block by 2."""
    Hd = W.shape[0]
    W4 = np.asarray(W, np.float64).reshape(Hd, 4, 512)
    out = np.empty_like(W4)
    out[:, 0] = W4[:, 0]
    out[:, 1] = 2.0 * W4[:, 2]
    out[:, 2] = W4[:, 1]
    out[:, 3] = W4[:, 3]
    return out.reshape(Hd, 2048)


def _wh8_arrange(W):
    # W: [512, 2048] f64 -> [128, 2*2*2048] fp8 with value at col (i*4096+j*2048+n)
    # = W[256i+128j+ki, n]
    W4 = W.reshape(2, 2, 128, 2048)  # [i, j, ki, n]
    arr = np.ascontiguousarray(np.transpose(W4, (2, 0, 1, 3)).reshape(128, 8192))
    return arr.astype(np_f8)


def _wh16_arrange(W):
    # W: [512, 2048] -> [128, 4*2048] bf16 with value at col (kc*2048+n) = W[kc*128+p, n]
    return np.ascontiguousarray(W.reshape(4, 128, 2048).transpose(1, 0, 2).reshape(128, 8192)).astype(np_bf16)


def _zx_rep(zx):
    # [16, 2048] -> [128, 2048] bf16 with the table replicated at rows 32j+v
    out = np.zeros((128, 2048), np.float64)
    for j in range(4):
        out[32 * j : 32 * j + VIN] = zx
    return out.astype(np_bf16)


def _onehot_table(tokens, S):
    # [128, S*128] bf16: row 32j+v, col t*128+b = (tokens[b, t] == v)
    oh = (np.arange(VIN)[:, None, None] == tokens.T[None, :, :])  # [16, S, 128b]
    arr = np.zeros((32, S, 128), np.float32)
    arr[:VIN] = oh
    flat = arr.reshape(32, S * 128)
    return np.tile(flat, (4, 1)).astype(np_bf16)


def make_in_maps(tokens, Wh_f, Wh_b, Wh_d, zx_f, zx_b, ze_d, W_out, b_out):
    B = tokens.shape[0]
    assert B % N_CORES == 0
    bl = B // N_CORES
    ident = np.eye(128, dtype=np.float32).astype(np_bf16)
    iota9 = np.tile(np.tile(np.arange(32, dtype=np.float32), 4), (128, 1))
    ones = np.ones((1, 128), np.float32).astype(np_bf16)
    oh0 = np.zeros((128, 128), np.float32)
    oh0[::32, :] = 1.0
    oh0 = oh0.astype(np_bf16)

    whf = _perm_scale(Wh_f)
    whb = _perm_scale(Wh_b)
    whd = _perm_scale(Wh_d)
    zedp = np.zeros((128, 2048), np.float64)
    zep = _perm_scale(ze_d)
    for j in range(4):
        zedp[32 * j : 32 * j + VOUT] = zep

    wout = np.zeros((128, 64), np.float32)
    Wov = np.asarray(W_out, np.float32).reshape(4, 128, VOUT)
    for kc in range(4):
        wout[:, kc * 16 : kc * 16 + VOUT] = Wov[kc]

    common = dict(
        wh16d=_wh16_arrange(whd),
        zxf=_zx_rep(_perm_scale(zx_f)[:VIN]),
        zxb=_zx_rep(_perm_scale(zx_b)[:VIN]),
        zed=zedp.astype(np_bf16),
        wout=wout.astype(np_bf16),
        bout=np.pad(np.asarray(b_out, np.float32).reshape(1, VOUT), ((0, 0), (0, 16 - VOUT))).astype(np_bf16),
        ident=ident, iota9=iota9, ones=ones, oh0=oh0,
    )
    # the program declares all weight tensors; feed them all (unused ones are
    # never DMA'd by the device program)
    common["wh8f"] = _wh8_arrange(whf)
    common["wh8b"] = _wh8_arrange(whb)
    common["wh8d"] = _wh8_arrange(whd)
    common["wh16f"] = _wh16_arrange(whf)
    common["wh16b"] = _wh16_arrange(whb)
    S = tokens.shape[1]
    return [
        {**common, "ohts": _onehot_table(tokens[c * bl : (c + 1) * bl], S)}
        for c in range(N_CORES)
    ]


def fold_tables(emb_in, Wi_f, b_f, Wi_b, b_b, emb_out, Wi_d, b_d):
    f8_ = lambda x: np.asarray(x, np.float64)
    zx_f = f8_(emb_in) @ f8_(Wi_f) + f8_(b_f)
    zx_b = f8_(emb_in) @ f8_(Wi_b) + f8_(b_b)
    ze_d = f8_(emb_out) @ f8_(Wi_d) + f8_(b_d)
    return zx_f, zx_b, ze_d


def kernel(tokens, emb_in, Wi_f, Wh_f, b_f, Wi_b, Wh_b, b_b,
           emb_out, Wi_d, Wh_d, b_d, W_out, b_out, max_length):
    T = int(max_length)
    tokens = np.asarray(tokens, np.int32)
    B, S = tokens.shape
    zx_f, zx_b, ze_d = fold_tables(emb_in, Wi_f, b_f, Wi_b, b_b, emb_out, Wi_d, b_d)
    nc = _get_program(S, T)
    in_maps = make_in_maps(tokens, Wh_f, Wh_b, Wh_d, zx_f, zx_b, ze_d, W_out, b_out)
    res = run_bass_kernel_spmd(nc, in_maps, list(range(N_CORES)))
    bl = B // N_CORES
    preds = np.concatenate(
        [res.results[c]["preds"].reshape(bl, T, VOUT) for c in range(N_CORES)], axis=0
    )
    return np.ascontiguousarray(preds, np.float32)
